# revision 25
# baseline (speedup 1.0000x reference)
"""Trainium2 Bass kernel for nn_Encoding (dense transformer block with
inter-attention + gated fusion), data-parallel over batch on 8 NeuronCores.

Reference math per batch b (P: [n, d], weights small):
  wa, wb, wc = split(w_itr_att)
  A[i,j]   = P[i].wb + P[j].wa + sum_d P[i,d]*wc[d]*P[j,d]
  SA       = softmax_j(A)
  itr      = SA @ P
  Pc       = [P, itr]
  z = tanh(Pc@w1+b1); r = sig(Pc@w2+b2); f = sig(Pc@w3+b3)
  out      = r*P + f*z

Structure:
  - exp(P[i].wb) cancels in softmax -> wb dropped.
  - Scores computed TRANSPOSED (At[j,i]); P[j].wa is a per-partition exp bias.
  - P^T and wc*P^T are prepared on the HOST (bf16) and DMA'd directly - no
    on-chip transposes or casts of P at all.  P itself is only loaded as fp8
    (itr-matmul stationary).
  - exp split between ACT (true exp, with a +(C-56)*ln2/8 bias so the global
    scale matches) and DVE via the fp8e4m3 Schraudolph bit trick
    exp(x) ~= bits(round(11.5416*x + C)) as a single tensor_scalar
    (mult + per-partition-column add) writing uint8 bitcast to fp8e4.  The
    C-vs-56 offset is a global scale on T that cancels in the softmax.
  - denominator via ones^T-DR-matmul; reciprocal on DVE; broadcast across
    partitions via a K=1 matmul; normalization fused into the PSUM->bf16
    evac of itr^T.
  - Gates computed TRANSPOSED (g^T[d_out, n]): per-gate [128,128] bf16
    stationaries; bias is a per-partition column fused into the ACT tanh;
    sigmoid(x) = 0.5 + 0.5*tanh(0.5*x) keeps the exp/tanh table set.
  - Output stays transposed: 2*out^T = (1+tanh_r)*P^T + (1+tanh_f)*z is
    written to DRAM transposed; the host applies the 0.5 and un-transposes.
  - The PE stream interleaves scores-matmuls of batch b (paced by exp) with
    attention/gate matmuls of batch b-1 so exp latency never idles the PE.
  - Row permutation n = p*8+t keeps every DMA descriptor contiguous.
"""
from contextlib import ExitStack

import numpy as np
import ml_dtypes

import concourse.bass as bass
import concourse.mybir as mybir
import concourse.tile as tile
import concourse.tile_sem_assignment as tsa
from concourse import bacc
from concourse.bass_utils import run_bass_kernel_spmd

tsa.NUM_HWDGE_SEMS = 1

B, N, D = 32, 1024, 128
NCORES = 8
BPC = B // NCORES          # batches per core
NB = N // 128              # 128-row blocks per batch
f32 = mybir.dt.float32
bf16 = mybir.dt.bfloat16
fp8 = mybir.dt.float8e4
u8 = mybir.dt.uint8
DR = mybir.MatmulPerfMode.DoubleRow
Exp = mybir.ActivationFunctionType.Exp
Tanh = mybir.ActivationFunctionType.Tanh
Mult = mybir.AluOpType.mult
Add = mybir.AluOpType.add

# Schraudolph-in-fp8e4m3: exp(x) ~= bits(round(M_SCHR*x + C_SCHR)).
M_SCHR = 11.5416
C_SCHR = 62.0
D_ACT = float((C_SCHR - 56.0) * np.log(2.0) / 8.0)   # ACT-path bias, scale match
# per-j-block exp engine: a=ACT (true exp), d=DVE (bit trick)
EXP_SPLIT = "adaadada"


class _State:
    pass


def _load(nc, s, bi):
    """DMA host-prepared P^T, wc*P^T (bf16) and P (fp8)."""
    pt_h = s.ld.tile([128, NB, 128], bf16, tag="pt_h")
    nc.sync.dma_start(out=pt_h, in_=s.P_t[bi].rearrange("d (t m) -> d t m", t=NB))
    pwct_h = s.ld.tile([128, NB, 128], bf16, tag="pwct_h")
    nc.vector.tensor_scalar_mul(pwct_h, pt_h, s.wc_col)
    pn_f8 = s.ld.tile([128, NB, 128], fp8, tag="pn_f8")
    nc.sync.dma_start(out=pn_f8, in_=s.P_f8[bi].rearrange("(p t) d -> p t d", t=NB))
    s.pt_h[bi], s.pwct_h[bi], s.pn_f8[bi] = pt_h, pwct_h, pn_f8


def _scores(nc, s, bi, chunks):
    """v = P.wa; scores At[j,i] per j-block; exp on ACT/DVE -> st (fp8).

    `chunks` are PE-work closures of the previous batch, interleaved
    between score blocks to fill the PE while exp paces the at-ring."""
    pt_h, pwct_h = s.pt_h[bi], s.pwct_h[bi]

    st = s.big.tile([128, NB, 1024], fp8, tag="st")
    st_u8 = st.bitcast(u8)
    ci = 0
    for jb in range(NB):
        at = s.ps_at.tile([128, 1024], f32, tag="at")
        nc.tensor.matmul(at[:, 0:512], pt_h[:, jb, :],
                         pwct_h[:, 0:4, :], start=True, stop=True)
        nc.tensor.matmul(at[:, 512:1024], pt_h[:, jb, :],
                         pwct_h[:, 4:8, :], start=True, stop=True)
        with s.tc.high_priority(offset=50):
            if EXP_SPLIT[jb] == "a":
                nc.scalar.activation(st[:, jb, :], at, Exp,
                                     bias=s.vcols[:, bi, jb:jb + 1])
            else:
                nc.vector.tensor_scalar(st_u8[:, jb, :], at, M_SCHR,
                                        s.vcols[:, bi, 8 + jb:9 + jb], Mult, Add)
        if ci < len(chunks):
            chunks[ci]()
            ci += 1
    for c in chunks[ci:]:
        c()
    s.st[bi] = st


def _attn_gates_chunks(nc, s, bi):
    """PE-work closures for attention + gates of batch bi (with their
    attached ACT/DVE/GPSIMD consumers), interleaved into the next batch's
    score stream.  The tail chain per half-batch is:
    den -> recip (DVE) -> partition_broadcast (GPSIMD) -> itr-norm (DVE)
    -> gate MMs -> tanh (ACT) -> fused output (DVE) -> DMA."""
    st, pn_f8, pt_h = s.st[bi], s.pn_f8[bi], s.pt_h[bi]
    h = {}

    def den_chunk(c):
        def go():
            den_ps = s.ps_di.tile([128, 512], f32, tag="di")
            cs = slice(c * 512, (c + 1) * 512)
            for t in range(NB // 2):
                nc.tensor.matmul(den_ps[0:1, :], s.ones_f8,
                                 st[:, 2 * t:2 * t + 2, cs],
                                 perf_mode=DR, start=(t == 0), stop=(t == 3))
            if c == 0:
                recip_f = s.work.tile([1, 1024], f32, tag="recip")
                bc_f = s.work.tile([128, NB, 128], f32, tag="bc_f")
                h["recip_f"], h["bc_f"] = recip_f, bc_f
            cs2 = slice(c * 512, (c + 1) * 512)
            nc.vector.reciprocal_approx_fast(h["recip_f"][:, cs2], den_ps[0:1, :])
            nc.gpsimd.partition_broadcast(
                h["bc_f"][:, 4 * c:4 * c + 4, :], h["recip_f"][:, cs2])
        return go

    def itr_chunk(c):
        def go():
            itr_ps = s.ps_di.tile([128, 512], f32, tag="di")
            cs = slice(c * 512, (c + 1) * 512)
            for t in range(NB // 2):
                nc.tensor.matmul(itr_ps, pn_f8[:, 2 * t:2 * t + 2, :],
                                 st[:, 2 * t:2 * t + 2, cs], perf_mode=DR,
                                 start=(t == 0), stop=(t == 3))
            if c == 0:
                itrt = s.work.tile([128, NB, 128], bf16, tag="itrt_h")
                h["itrt"] = itrt
            with nc.allow_low_precision(reason="bf16 itr"):
                nc.vector.tensor_mul(
                    h["itrt"][:, 4 * c:4 * c + 4, :],
                    itr_ps.rearrange("p (t d) -> p t d", t=4),
                    h["bc_f"][:, 4 * c:4 * c + 4, :])
        return go

    def gate_chunk(c, g):
        def go():
            if c == 0 and g == 0:
                zt = s.work.tile([128, NB, 128], bf16, tag="zt")
                tr = s.work.tile([128, NB, 128], bf16, tag="tr")
                tf = s.work.tile([128, NB, 128], bf16, tag="tf")
                h["zt"], h["tr"], h["tf"] = zt, tr, tf
            hb = slice(4 * c, 4 * c + 4)
            g_ps = s.ps_g.tile([128, 512], f32, tag="g")
            nc.tensor.matmul(g_ps, s.w_h[:, g, 0, :], pt_h[:, hb, :],
                             start=True, stop=False)
            nc.tensor.matmul(g_ps, s.w_h[:, g, 1, :], h["itrt"][:, hb, :],
                             start=False, stop=True)
            if g == 0:
                nc.scalar.activation(h["zt"][:, hb, :], g_ps, Tanh,
                                     bias=s.b_cols[:, 0:1])
            else:
                key = "tr" if g == 1 else "tf"
                nc.scalar.activation(h[key][:, hb, :], g_ps, Tanh,
                                     bias=s.bhalf[:, g:g + 1], scale=0.5)
        return go

    def u_chunk(c):
        def go():
            hb = slice(4 * c, 4 * c + 4)
            if c == 0:
                u = s.work.tile([128, NB, 128], bf16, tag="u")
                h["u"] = u
            nc.vector.scalar_tensor_tensor(h["u"][:, hb, :], h["tr"][:, hb, :],
                                           1.0, pt_h[:, hb, :], Add, Mult)
        return go

    def out_chunk(c):
        def go():
            hb = slice(4 * c, 4 * c + 4)
            # 2*out^T = (1+tr)*P^T + (1+tf)*z; host halves + un-transposes
            if c == 0:
                w2t = s.work.tile([128, NB, 128], bf16, tag="w2t")
                out_th = s.work.tile([128, NB, 128], bf16, tag="out_th")
                h["w2t"], h["out_th"] = w2t, out_th
            nc.vector.scalar_tensor_tensor(h["w2t"][:, hb, :], h["tf"][:, hb, :],
                                           1.0, h["zt"][:, hb, :], Add, Mult)
            nc.vector.tensor_add(h["out_th"][:, hb, :], h["u"][:, hb, :],
                                 h["w2t"][:, hb, :])
            nc.sync.dma_start(
                out=s.out[bi].rearrange("d (t m) -> d t m", t=NB)[:, hb, :],
                in_=h["out_th"][:, hb, :])
        return go

    def fuse(*fns):
        def go():
            for f in fns:
                f()
        return go

    return [
        den_chunk(0),
        itr_chunk(0),
        den_chunk(1),
        itr_chunk(1),
        gate_chunk(0, 0),
        fuse(gate_chunk(0, 1), gate_chunk(0, 2)),
        fuse(u_chunk(0), gate_chunk(1, 0), out_chunk(0)),
        fuse(gate_chunk(1, 1), gate_chunk(1, 2)),
        fuse(u_chunk(1), out_chunk(1)),
    ]


def _body(nc, tc, ctx):
    s = _State()
    s.tc = tc
    s.P_t = nc.dram_tensor("P_t", [BPC, D, N], bf16, kind="ExternalInput")
    s.P_wct = nc.dram_tensor("P_wct", [BPC, D, N], bf16, kind="ExternalInput")
    s.P_f8 = nc.dram_tensor("P_f8", [BPC, N, D], fp8, kind="ExternalInput")
    w_att = nc.dram_tensor("w_itr_att", [3 * D], f32, kind="ExternalInput")
    w1 = nc.dram_tensor("w1", [2 * D, D], f32, kind="ExternalInput")
    w2 = nc.dram_tensor("w2", [2 * D, D], f32, kind="ExternalInput")
    w3 = nc.dram_tensor("w3", [2 * D, D], f32, kind="ExternalInput")
    b1 = nc.dram_tensor("b1", [D], f32, kind="ExternalInput")
    b2 = nc.dram_tensor("b2", [D], f32, kind="ExternalInput")
    b3 = nc.dram_tensor("b3", [D], f32, kind="ExternalInput")
    s.vcols_d = nc.dram_tensor("vcols", [128, BPC * 16], f32, kind="ExternalInput")
    s.out = nc.dram_tensor("out", [BPC, D, N], bf16, kind="ExternalOutput")

    singles = ctx.enter_context(tc.tile_pool(name="singles", bufs=1))
    s.pt_h, s.pwct_h, s.pn_f8, s.st = {}, {}, {}, {}

    s.work = ctx.enter_context(tc.tile_pool(name="work", bufs=3))
    s.ld = ctx.enter_context(tc.tile_pool(name="ld", bufs=3))
    s.big = ctx.enter_context(tc.tile_pool(name="big", bufs=3))
    # PSUM: ps_at 2x[128,1024] (4) + ps_di 2x[128,512] (2) + ps_g 2x[128,512]
    # (2) = 8 banks
    s.ps_at = ctx.enter_context(tc.tile_pool(name="ps_at", bufs=2, space="PSUM"))
    s.ps_di = ctx.enter_context(tc.tile_pool(name="ps_di", bufs=2, space="PSUM"))
    s.ps_g = ctx.enter_context(tc.tile_pool(name="ps_g", bufs=2, space="PSUM"))
    vc_t = singles.tile([128, BPC, 16], f32)
    nc.sync.dma_start(out=vc_t,
                      in_=s.vcols_d[0:128, :].rearrange("p (b c) -> p b c", b=BPC))
    s.vcols = vc_t
    watt_row = singles.tile([1, 3 * D], f32)
    nc.sync.dma_start(out=watt_row, in_=w_att.rearrange("(o c) -> o c", o=1))
    ones2_w = singles.tile([1, 2], f32)
    nc.vector.memset(ones2_w, 1.0)
    wc_ps = s.ps_g.tile([128, 512], f32, tag="g")
    nc.tensor.matmul(wc_ps[:, 0:2], watt_row[:, 256:384], ones2_w,
                     start=True, stop=True)
    s.wc_col = singles.tile([128, 1], f32)
    nc.vector.tensor_copy(s.wc_col, wc_ps[:, 0:1])
    _load(nc, s, 0)

    # ---- constants ----
    ones2_f = singles.tile([1, 2], f32)
    nc.vector.memset(ones2_f, 1.0)

    ones_f = singles.tile([128, 1], f32)
    nc.vector.memset(ones_f, 1.0)
    ones_rf = singles.tile([1, 128], f32)
    nc.vector.memset(ones_rf, 1.0)
    # DoubleRow denominator lhsT: Ko-dim byte step must be %16 == 0
    ones_f8_pad = singles.tile([128, 2, 16], fp8)
    nc.vector.tensor_copy(ones_f8_pad[:, 0, 0:1], ones_f)
    nc.vector.tensor_copy(ones_f8_pad[:, 1, 0:1], ones_f)
    s.ones_f8 = ones_f8_pad[:, :, 0:1]

    # Gate weights: per-gate stationaries [d, 128]: rows 0:128 (vs P^T) and
    # rows 128:256 (vs itr^T).
    wstage = singles.tile([128, 3, 2, 128], f32)
    for gi, w in enumerate((w1, w2, w3)):
        nc.gpsimd.dma_start(out=wstage[:, gi, 0, :], in_=w[0:128, :])
        nc.gpsimd.dma_start(out=wstage[:, gi, 1, :], in_=w[128:256, :])
    s.w_h = singles.tile([128, 3, 2, 128], bf16)
    nc.vector.tensor_copy(s.w_h, wstage)

    # biases as per-partition columns (rows -> columns via K=1 matmuls)
    bstage = singles.tile([1, 3, 128], f32)
    for gi, bvec in enumerate((b1, b2, b3)):
        nc.gpsimd.dma_start(out=bstage[:, gi, :],
                            in_=bvec.rearrange("(o p) -> o p", o=1))
    b_ps = s.ps_g.tile([128, 512], f32, tag="g")
    for gi in range(3):
        nc.tensor.matmul(b_ps[:, 2 * gi:2 * gi + 2], bstage[:, gi, :], ones2_f,
                         start=True, stop=True)
    s.b_cols = singles.tile([128, 3], f32)
    nc.vector.tensor_copy(s.b_cols, b_ps[:, 0:6:2])
    s.bhalf = singles.tile([128, 3], f32)
    nc.vector.tensor_scalar_mul(s.bhalf, s.b_cols, 0.5)

    # Pipeline: batch bi's scores interleave the PE chunks of bi-1.
    _load(nc, s, 1)
    chunks = []
    for bi in range(BPC):
        if bi + 2 < BPC:
            _load(nc, s, bi + 2)
        _scores(nc, s, bi, chunks)
        chunks = _attn_gates_chunks(nc, s, bi)
    for c in chunks:
        c()


_NC_CACHE = {}


def _get_nc():
    if "nc" not in _NC_CACHE:
        nc = bacc.Bacc(None)
        with tile.TileContext(nc) as tc:
            with ExitStack() as ctx:
                _body(nc, tc, ctx)
        nc.finalize()
        _NC_CACHE["nc"] = nc
    return _NC_CACHE["nc"]


def _prep_inputs(inputs):
    P = np.asarray(inputs["P"], dtype=np.float32)
    watt = np.asarray(inputs["w_itr_att"], dtype=np.float32)
    wa, wc = watt[:D], watt[2 * D:]
    in_maps = []
    for c in range(NCORES):
        shard = np.ascontiguousarray(P[c * BPC:(c + 1) * BPC])  # [BPC, N, D]
        # transposed, column-permuted: cols (t, m) <-> n = m*NB + t
        pt = shard.transpose(0, 2, 1).reshape(BPC, D, 128, NB)
        pt = np.ascontiguousarray(pt.transpose(0, 1, 3, 2)).reshape(BPC, D, N)
        v = (shard @ wa).reshape(BPC, 128, NB)        # v[bi, p, t], n = p*NB+t
        vcols = np.concatenate([v + D_ACT, M_SCHR * v + C_SCHR], axis=2)
        vcols = np.ascontiguousarray(vcols.transpose(1, 0, 2)).reshape(128, BPC * 16)
        m = {
            "P_t": pt.astype(ml_dtypes.bfloat16),
            "vcols": vcols.astype(np.float32),
            "P_wct": (wc[None, :, None] * pt).astype(ml_dtypes.bfloat16),
            "P_f8": shard.astype(ml_dtypes.float8_e4m3),
            "w_itr_att": np.asarray(inputs["w_itr_att"], dtype=np.float32),
            "w1": np.asarray(inputs["w1"], dtype=np.float32),
            "w2": np.asarray(inputs["w2"], dtype=np.float32),
            "w3": np.asarray(inputs["w3"], dtype=np.float32),
            "b1": np.asarray(inputs["b1"], dtype=np.float32),
            "b2": np.asarray(inputs["b2"], dtype=np.float32),
            "b3": np.asarray(inputs["b3"], dtype=np.float32),
        }
        in_maps.append(m)
    return in_maps


def _run(inputs, _retries=2, **kw):
    nc = _get_nc()
    in_maps = _prep_inputs(inputs)
    import time
    for attempt in range(_retries + 1):
        try:
            res = run_bass_kernel_spmd(nc, in_maps,
                                       core_ids=list(range(NCORES)), **kw)
            break
        except Exception:  # wedged device from a prior aborted run
            if attempt == _retries:
                raise
            time.sleep(20)
    shards = []
    for r in res.results:
        a = np.asarray(r["out"]).astype(np.float32)        # [BPC, D, (t, m)]
        a = a.reshape(BPC, D, NB, 128).transpose(0, 3, 2, 1)  # -> [BPC, m, t, D]
        shards.append(a.reshape(BPC, N, D))                # n = m*NB + t
    outp = 0.5 * np.concatenate(shards, axis=0)
    return outp.astype(np.float32), res


def kernel(**inputs):
    out, _ = _run(inputs)
    return out


# revision 27
# speedup vs baseline: 1.0070x; 1.0070x over previous
"""Trainium2 Bass kernel for nn_Encoding (dense transformer block with
inter-attention + gated fusion), data-parallel over batch on 8 NeuronCores.

Reference math per batch b (P: [n, d], weights small):
  wa, wb, wc = split(w_itr_att)
  A[i,j]   = P[i].wb + P[j].wa + sum_d P[i,d]*wc[d]*P[j,d]
  SA       = softmax_j(A)
  itr      = SA @ P
  Pc       = [P, itr]
  z = tanh(Pc@w1+b1); r = sig(Pc@w2+b2); f = sig(Pc@w3+b3)
  out      = r*P + f*z

Structure:
  - exp(P[i].wb) cancels in softmax -> wb dropped.
  - Scores computed TRANSPOSED (At[j,i]); P[j].wa is a per-partition exp bias.
  - P^T and wc*P^T are prepared on the HOST (bf16) and DMA'd directly - no
    on-chip transposes or casts of P at all.  P itself is only loaded as fp8
    (itr-matmul stationary).
  - exp split between ACT (true exp, with a +(C-56)*ln2/8 bias so the global
    scale matches) and DVE via the fp8e4m3 Schraudolph bit trick
    exp(x) ~= bits(round(11.5416*x + C)) as a single tensor_scalar
    (mult + per-partition-column add) writing uint8 bitcast to fp8e4.  The
    C-vs-56 offset is a global scale on T that cancels in the softmax.
  - denominator via ones^T-DR-matmul; reciprocal on DVE; broadcast across
    partitions via a K=1 matmul; normalization fused into the PSUM->bf16
    evac of itr^T.
  - Gates computed TRANSPOSED (g^T[d_out, n]): per-gate [128,128] bf16
    stationaries; bias is a per-partition column fused into the ACT tanh;
    sigmoid(x) = 0.5 + 0.5*tanh(0.5*x) keeps the exp/tanh table set.
  - Output stays transposed: 2*out^T = (1+tanh_r)*P^T + (1+tanh_f)*z is
    written to DRAM transposed; the host applies the 0.5 and un-transposes.
  - The PE stream interleaves scores-matmuls of batch b (paced by exp) with
    attention/gate matmuls of batch b-1 so exp latency never idles the PE.
  - Row permutation n = p*8+t keeps every DMA descriptor contiguous.
"""
from contextlib import ExitStack

import numpy as np
import ml_dtypes

import concourse.bass as bass
import concourse.mybir as mybir
import concourse.tile as tile
import concourse.tile_sem_assignment as tsa
from concourse import bacc
from concourse.bass_utils import run_bass_kernel_spmd

tsa.NUM_HWDGE_SEMS = 1

B, N, D = 32, 1024, 128
NCORES = 8
BPC = B // NCORES          # batches per core
NB = N // 128              # 128-row blocks per batch
f32 = mybir.dt.float32
bf16 = mybir.dt.bfloat16
fp8 = mybir.dt.float8e4
u8 = mybir.dt.uint8
DR = mybir.MatmulPerfMode.DoubleRow
Exp = mybir.ActivationFunctionType.Exp
Tanh = mybir.ActivationFunctionType.Tanh
Mult = mybir.AluOpType.mult
Add = mybir.AluOpType.add

# Schraudolph-in-fp8e4m3: exp(x) ~= bits(round(M_SCHR*x + C_SCHR)).
M_SCHR = 11.5416
C_SCHR = 62.0
D_ACT = float((C_SCHR - 56.0) * np.log(2.0) / 8.0)   # ACT-path bias, scale match
# per-j-block exp engine: a=ACT (true exp), d=DVE (bit trick)
EXP_SPLIT = "adaadada"


class _State:
    pass


def _load(nc, s, bi):
    """DMA host-prepared P^T, wc*P^T (bf16) and P (fp8)."""
    pt_h = s.ld.tile([128, NB, 128], bf16, tag="pt_h")
    nc.sync.dma_start(out=pt_h, in_=s.P_t[bi].rearrange("d (t m) -> d t m", t=NB))
    pwct_h = s.ld.tile([128, NB, 128], bf16, tag="pwct_h")
    nc.sync.dma_start(out=pwct_h,
                      in_=s.P_wct[bi].rearrange("d (t m) -> d t m", t=NB))
    pn_f8 = s.ld.tile([128, NB, 128], fp8, tag="pn_f8")
    nc.sync.dma_start(out=pn_f8, in_=s.P_f8[bi].rearrange("(p t) d -> p t d", t=NB))
    s.pt_h[bi], s.pwct_h[bi], s.pn_f8[bi] = pt_h, pwct_h, pn_f8


def _scores(nc, s, bi, chunks):
    """v = P.wa; scores At[j,i] per j-block; exp on ACT/DVE -> st (fp8).

    `chunks` are PE-work closures of the previous batch, interleaved
    between score blocks to fill the PE while exp paces the at-ring."""
    pt_h, pwct_h = s.pt_h[bi], s.pwct_h[bi]

    st = s.big.tile([128, NB, 1024], fp8, tag="st")
    st_u8 = st.bitcast(u8)
    ci = 0
    for jb in range(NB):
        at = s.ps_at.tile([128, 1024], f32, tag="at")
        nc.tensor.matmul(at[:, 0:512], pt_h[:, jb, :],
                         pwct_h[:, 0:4, :], start=True, stop=True)
        nc.tensor.matmul(at[:, 512:1024], pt_h[:, jb, :],
                         pwct_h[:, 4:8, :], start=True, stop=True)
        with s.tc.high_priority(offset=50):
            if EXP_SPLIT[jb] == "a":
                nc.scalar.activation(st[:, jb, :], at, Exp,
                                     bias=s.vcols[:, bi, jb:jb + 1])
            else:
                nc.vector.tensor_scalar(st_u8[:, jb, :], at, M_SCHR,
                                        s.vcols[:, bi, 8 + jb:9 + jb], Mult, Add)
        if ci < len(chunks):
            chunks[ci]()
            ci += 1
    for c in chunks[ci:]:
        c()
    s.st[bi] = st


def _attn_gates_chunks(nc, s, bi):
    """PE-work closures for attention + gates of batch bi (with their
    attached ACT/DVE/GPSIMD consumers), interleaved into the next batch's
    score stream.  The tail chain per half-batch is:
    den -> recip (DVE) -> partition_broadcast (GPSIMD) -> itr-norm (DVE)
    -> gate MMs -> tanh (ACT) -> fused output (DVE) -> DMA."""
    st, pn_f8, pt_h = s.st[bi], s.pn_f8[bi], s.pt_h[bi]
    h = {}

    def den_chunk(c):
        def go():
            den_ps = s.ps_di.tile([128, 512], f32, tag="di")
            cs = slice(c * 512, (c + 1) * 512)
            for t in range(NB // 2):
                nc.tensor.matmul(den_ps[0:1, :], s.ones_f8,
                                 st[:, 2 * t:2 * t + 2, cs],
                                 perf_mode=DR, start=(t == 0), stop=(t == 3))
            if c == 0:
                recip_f = s.work.tile([1, 1024], f32, tag="recip")
                bc_f = s.work.tile([128, NB, 128], f32, tag="bc_f")
                h["recip_f"], h["bc_f"] = recip_f, bc_f
            cs2 = slice(c * 512, (c + 1) * 512)
            nc.vector.reciprocal_approx_fast(h["recip_f"][:, cs2], den_ps[0:1, :])
            nc.gpsimd.partition_broadcast(
                h["bc_f"][:, 4 * c:4 * c + 4, :], h["recip_f"][:, cs2])
        return go

    def itr_chunk(c):
        def go():
            itr_ps = s.ps_di.tile([128, 512], f32, tag="di")
            cs = slice(c * 512, (c + 1) * 512)
            for t in range(NB // 2):
                nc.tensor.matmul(itr_ps, pn_f8[:, 2 * t:2 * t + 2, :],
                                 st[:, 2 * t:2 * t + 2, cs], perf_mode=DR,
                                 start=(t == 0), stop=(t == 3))
            if c == 0:
                itrt = s.work.tile([128, NB, 128], bf16, tag="itrt_h")
                h["itrt"] = itrt
            with nc.allow_low_precision(reason="bf16 itr"):
                nc.vector.tensor_mul(
                    h["itrt"][:, 4 * c:4 * c + 4, :],
                    itr_ps.rearrange("p (t d) -> p t d", t=4),
                    h["bc_f"][:, 4 * c:4 * c + 4, :])
        return go

    def gate_chunk(c, g):
        def go():
            if c == 0 and g == 0:
                zt = s.work.tile([128, NB, 128], bf16, tag="zt")
                tr = s.work.tile([128, NB, 128], bf16, tag="tr")
                tf = s.work.tile([128, NB, 128], bf16, tag="tf")
                h["zt"], h["tr"], h["tf"] = zt, tr, tf
            hb = slice(4 * c, 4 * c + 4)
            g_ps = s.ps_g.tile([128, 512], f32, tag="g")
            nc.tensor.matmul(g_ps, s.w_h[:, g, 0, :], pt_h[:, hb, :],
                             start=True, stop=False)
            nc.tensor.matmul(g_ps, s.w_h[:, g, 1, :], h["itrt"][:, hb, :],
                             start=False, stop=True)
            if g == 0:
                nc.scalar.activation(h["zt"][:, hb, :], g_ps, Tanh,
                                     bias=s.b_cols[:, 0:1])
            else:
                key = "tr" if g == 1 else "tf"
                nc.scalar.activation(h[key][:, hb, :], g_ps, Tanh,
                                     bias=s.bhalf[:, g:g + 1], scale=0.5)
        return go

    def u_chunk(c):
        def go():
            hb = slice(4 * c, 4 * c + 4)
            if c == 0:
                u = s.work.tile([128, NB, 128], bf16, tag="u")
                h["u"] = u
            nc.vector.scalar_tensor_tensor(h["u"][:, hb, :], h["tr"][:, hb, :],
                                           1.0, pt_h[:, hb, :], Add, Mult)
        return go

    def out_chunk(c):
        def go():
            hb = slice(4 * c, 4 * c + 4)
            # 2*out^T = (1+tr)*P^T + (1+tf)*z; host halves + un-transposes
            if c == 0:
                w2t = s.work.tile([128, NB, 128], bf16, tag="w2t")
                out_th = s.work.tile([128, NB, 128], bf16, tag="out_th")
                h["w2t"], h["out_th"] = w2t, out_th
            nc.vector.scalar_tensor_tensor(h["w2t"][:, hb, :], h["tf"][:, hb, :],
                                           1.0, h["zt"][:, hb, :], Add, Mult)
            nc.vector.tensor_add(h["out_th"][:, hb, :], h["u"][:, hb, :],
                                 h["w2t"][:, hb, :])
            nc.sync.dma_start(
                out=s.out[bi].rearrange("d (t m) -> d t m", t=NB)[:, hb, :],
                in_=h["out_th"][:, hb, :])
        return go

    def fuse(*fns):
        def go():
            for f in fns:
                f()
        return go

    return [
        den_chunk(0),
        itr_chunk(0),
        den_chunk(1),
        itr_chunk(1),
        gate_chunk(0, 0),
        fuse(gate_chunk(0, 1), gate_chunk(0, 2)),
        fuse(u_chunk(0), gate_chunk(1, 0), out_chunk(0)),
        fuse(gate_chunk(1, 1), gate_chunk(1, 2)),
        fuse(u_chunk(1), out_chunk(1)),
    ]


def _body(nc, tc, ctx):
    s = _State()
    s.tc = tc
    s.P_t = nc.dram_tensor("P_t", [BPC, D, N], bf16, kind="ExternalInput")
    s.P_wct = nc.dram_tensor("P_wct", [BPC, D, N], bf16, kind="ExternalInput")
    s.P_f8 = nc.dram_tensor("P_f8", [BPC, N, D], fp8, kind="ExternalInput")
    w_att = nc.dram_tensor("w_itr_att", [3 * D], f32, kind="ExternalInput")
    w1 = nc.dram_tensor("w1", [2 * D, D], f32, kind="ExternalInput")
    w2 = nc.dram_tensor("w2", [2 * D, D], f32, kind="ExternalInput")
    w3 = nc.dram_tensor("w3", [2 * D, D], f32, kind="ExternalInput")
    b1 = nc.dram_tensor("b1", [D], f32, kind="ExternalInput")
    b2 = nc.dram_tensor("b2", [D], f32, kind="ExternalInput")
    b3 = nc.dram_tensor("b3", [D], f32, kind="ExternalInput")
    s.vcols_d = nc.dram_tensor("vcols", [128, BPC * 16], f32, kind="ExternalInput")
    s.out = nc.dram_tensor("out", [BPC, D, N], bf16, kind="ExternalOutput")

    singles = ctx.enter_context(tc.tile_pool(name="singles", bufs=1))
    s.pt_h, s.pwct_h, s.pn_f8, s.st = {}, {}, {}, {}

    s.work = ctx.enter_context(tc.tile_pool(name="work", bufs=3))
    s.ld = ctx.enter_context(tc.tile_pool(name="ld", bufs=3))
    s.big = ctx.enter_context(tc.tile_pool(name="big", bufs=3))
    # PSUM: ps_at 3x[128,1024] (6) + ps_di 1x[128,512] + ps_g 1x[128,512]
    # = 8 banks
    s.ps_at = ctx.enter_context(tc.tile_pool(name="ps_at", bufs=3, space="PSUM"))
    s.ps_di = ctx.enter_context(tc.tile_pool(name="ps_di", bufs=1, space="PSUM"))
    s.ps_g = ctx.enter_context(tc.tile_pool(name="ps_g", bufs=1, space="PSUM"))
    vc_t = singles.tile([128, BPC, 16], f32)
    nc.sync.dma_start(out=vc_t,
                      in_=s.vcols_d[0:128, :].rearrange("p (b c) -> p b c", b=BPC))
    s.vcols = vc_t
    _load(nc, s, 0)

    # ---- constants ----
    ones2_f = singles.tile([1, 2], f32)
    nc.vector.memset(ones2_f, 1.0)

    ones_f = singles.tile([128, 1], f32)
    nc.vector.memset(ones_f, 1.0)
    ones_rf = singles.tile([1, 128], f32)
    nc.vector.memset(ones_rf, 1.0)
    # DoubleRow denominator lhsT: Ko-dim byte step must be %16 == 0
    ones_f8_pad = singles.tile([128, 2, 16], fp8)
    nc.vector.tensor_copy(ones_f8_pad[:, 0, 0:1], ones_f)
    nc.vector.tensor_copy(ones_f8_pad[:, 1, 0:1], ones_f)
    s.ones_f8 = ones_f8_pad[:, :, 0:1]

    # Gate weights: per-gate stationaries [d, 128]: rows 0:128 (vs P^T) and
    # rows 128:256 (vs itr^T).
    wstage = singles.tile([128, 3, 2, 128], f32)
    for gi, w in enumerate((w1, w2, w3)):
        nc.gpsimd.dma_start(out=wstage[:, gi, 0, :], in_=w[0:128, :])
        nc.gpsimd.dma_start(out=wstage[:, gi, 1, :], in_=w[128:256, :])
    s.w_h = singles.tile([128, 3, 2, 128], bf16)
    nc.vector.tensor_copy(s.w_h, wstage)

    # biases as per-partition columns (rows -> columns via K=1 matmuls)
    bstage = singles.tile([1, 3, 128], f32)
    for gi, bvec in enumerate((b1, b2, b3)):
        nc.gpsimd.dma_start(out=bstage[:, gi, :],
                            in_=bvec.rearrange("(o p) -> o p", o=1))
    b_ps = s.ps_g.tile([128, 512], f32, tag="g")
    for gi in range(3):
        nc.tensor.matmul(b_ps[:, 2 * gi:2 * gi + 2], bstage[:, gi, :], ones2_f,
                         start=True, stop=True)
    s.b_cols = singles.tile([128, 3], f32)
    nc.vector.tensor_copy(s.b_cols, b_ps[:, 0:6:2])
    s.bhalf = singles.tile([128, 3], f32)
    nc.vector.tensor_scalar_mul(s.bhalf, s.b_cols, 0.5)

    # Pipeline: batch bi's scores interleave the PE chunks of bi-1.
    _load(nc, s, 1)
    chunks = []
    for bi in range(BPC):
        if bi + 2 < BPC:
            _load(nc, s, bi + 2)
        _scores(nc, s, bi, chunks)
        chunks = _attn_gates_chunks(nc, s, bi)
    for c in chunks:
        c()


_NC_CACHE = {}


def _get_nc():
    if "nc" not in _NC_CACHE:
        nc = bacc.Bacc(None)
        with tile.TileContext(nc) as tc:
            with ExitStack() as ctx:
                _body(nc, tc, ctx)
        nc.finalize()
        _NC_CACHE["nc"] = nc
    return _NC_CACHE["nc"]


def _prep_inputs(inputs):
    P = np.asarray(inputs["P"], dtype=np.float32)
    watt = np.asarray(inputs["w_itr_att"], dtype=np.float32)
    wa, wc = watt[:D], watt[2 * D:]
    in_maps = []
    for c in range(NCORES):
        shard = np.ascontiguousarray(P[c * BPC:(c + 1) * BPC])  # [BPC, N, D]
        # transposed, column-permuted: cols (t, m) <-> n = m*NB + t
        pt = shard.transpose(0, 2, 1).reshape(BPC, D, 128, NB)
        pt = np.ascontiguousarray(pt.transpose(0, 1, 3, 2)).reshape(BPC, D, N)
        v = (shard @ wa).reshape(BPC, 128, NB)        # v[bi, p, t], n = p*NB+t
        vcols = np.concatenate([v + D_ACT, M_SCHR * v + C_SCHR], axis=2)
        vcols = np.ascontiguousarray(vcols.transpose(1, 0, 2)).reshape(128, BPC * 16)
        m = {
            "P_t": pt.astype(ml_dtypes.bfloat16),
            "vcols": vcols.astype(np.float32),
            "P_wct": (wc[None, :, None] * pt).astype(ml_dtypes.bfloat16),
            "P_f8": shard.astype(ml_dtypes.float8_e4m3),
            "w_itr_att": np.asarray(inputs["w_itr_att"], dtype=np.float32),
            "w1": np.asarray(inputs["w1"], dtype=np.float32),
            "w2": np.asarray(inputs["w2"], dtype=np.float32),
            "w3": np.asarray(inputs["w3"], dtype=np.float32),
            "b1": np.asarray(inputs["b1"], dtype=np.float32),
            "b2": np.asarray(inputs["b2"], dtype=np.float32),
            "b3": np.asarray(inputs["b3"], dtype=np.float32),
        }
        in_maps.append(m)
    return in_maps


def _run(inputs, _retries=2, **kw):
    nc = _get_nc()
    in_maps = _prep_inputs(inputs)
    import time
    for attempt in range(_retries + 1):
        try:
            res = run_bass_kernel_spmd(nc, in_maps,
                                       core_ids=list(range(NCORES)), **kw)
            break
        except Exception:  # wedged device from a prior aborted run
            if attempt == _retries:
                raise
            time.sleep(20)
    shards = []
    for r in res.results:
        a = np.asarray(r["out"]).astype(np.float32)        # [BPC, D, (t, m)]
        a = a.reshape(BPC, D, NB, 128).transpose(0, 3, 2, 1)  # -> [BPC, m, t, D]
        shards.append(a.reshape(BPC, N, D))                # n = m*NB + t
    outp = 0.5 * np.concatenate(shards, axis=0)
    return outp.astype(np.float32), res


def kernel(**inputs):
    out, _ = _run(inputs)
    return out


# revision 29
# speedup vs baseline: 1.0566x; 1.0493x over previous
"""Trainium2 Bass kernel for nn_Encoding (dense transformer block with
inter-attention + gated fusion), data-parallel over batch on 8 NeuronCores.

Reference math per batch b (P: [n, d], weights small):
  wa, wb, wc = split(w_itr_att)
  A[i,j]   = P[i].wb + P[j].wa + sum_d P[i,d]*wc[d]*P[j,d]
  SA       = softmax_j(A)
  itr      = SA @ P
  Pc       = [P, itr]
  z = tanh(Pc@w1+b1); r = sig(Pc@w2+b2); f = sig(Pc@w3+b3)
  out      = r*P + f*z

Structure:
  - exp(P[i].wb) cancels in softmax -> wb dropped.
  - Scores computed TRANSPOSED (At[j,i]); P[j].wa is a per-partition exp bias.
  - P^T and wc*P^T are prepared on the HOST (bf16) and DMA'd directly - no
    on-chip transposes or casts of P at all.  P itself is only loaded as fp8
    (itr-matmul stationary).
  - exp split between ACT (true exp, with a +(C-56)*ln2/8 bias so the global
    scale matches) and DVE via the fp8e4m3 Schraudolph bit trick
    exp(x) ~= bits(round(11.5416*x + C)) as a single tensor_scalar
    (mult + per-partition-column add) writing uint8 bitcast to fp8e4.  The
    C-vs-56 offset is a global scale on T that cancels in the softmax.
  - denominator via ones^T-DR-matmul; reciprocal on DVE; broadcast across
    partitions via a K=1 matmul; normalization fused into the PSUM->bf16
    evac of itr^T.
  - Gates computed TRANSPOSED (g^T[d_out, n]): per-gate [128,128] bf16
    stationaries; bias is a per-partition column fused into the ACT tanh;
    sigmoid(x) = 0.5 + 0.5*tanh(0.5*x) keeps the exp/tanh table set.
  - Output stays transposed: 2*out^T = (1+tanh_r)*P^T + (1+tanh_f)*z is
    written to DRAM transposed; the host applies the 0.5 and un-transposes.
  - The PE stream interleaves scores-matmuls of batch b (paced by exp) with
    attention/gate matmuls of batch b-1 so exp latency never idles the PE.
  - Row permutation n = p*8+t keeps every DMA descriptor contiguous.
"""
from contextlib import ExitStack

import numpy as np
import ml_dtypes

import concourse.bass as bass
import concourse.mybir as mybir
import concourse.tile as tile
import concourse.tile_sem_assignment as tsa
from concourse import bacc
from concourse.bass_utils import run_bass_kernel_spmd

tsa.NUM_HWDGE_SEMS = 1

B, N, D = 32, 1024, 128
NCORES = 8
BPC = B // NCORES          # batches per core
NB = N // 128              # 128-row blocks per batch
f32 = mybir.dt.float32
bf16 = mybir.dt.bfloat16
fp8 = mybir.dt.float8e4
u8 = mybir.dt.uint8
DR = mybir.MatmulPerfMode.DoubleRow
Exp = mybir.ActivationFunctionType.Exp
Tanh = mybir.ActivationFunctionType.Tanh
Mult = mybir.AluOpType.mult
Add = mybir.AluOpType.add

# Schraudolph-in-fp8e4m3: exp(x) ~= bits(round(M_SCHR*x + C_SCHR)).
M_SCHR = 11.5416
C_SCHR = 62.0
D_ACT = float((C_SCHR - 56.0) * np.log(2.0) / 8.0)   # ACT-path bias, scale match
# per-j-block exp engine: a=ACT (true exp), d=DVE (bit trick)
EXP_SPLIT = "adaadada"


class _State:
    pass


def _load(nc, s, bi):
    """DMA host-prepared P^T and wc*P^T (bf16)."""
    pt_h = s.ld.tile([128, NB, 128], bf16, tag="pt_h")
    nc.sync.dma_start(out=pt_h, in_=s.P_t[bi].rearrange("d (t m) -> d t m", t=NB))
    pwct_h = s.ld.tile([128, NB, 128], bf16, tag="pwct_h")
    nc.sync.dma_start(out=pwct_h,
                      in_=s.P_wct[bi].rearrange("d (t m) -> d t m", t=NB))
    s.pt_h[bi], s.pwct_h[bi] = pt_h, pwct_h


def _load_f8(nc, s, bi):
    """DMA P (fp8, itr stationary) - only needed by the attention phase."""
    pn_f8 = s.ld.tile([128, NB, 128], fp8, tag="pn_f8")
    nc.sync.dma_start(out=pn_f8, in_=s.P_f8[bi].rearrange("(p t) d -> p t d", t=NB))
    s.pn_f8[bi] = pn_f8


def _scores(nc, s, bi, chunks):
    """v = P.wa; scores At[j,i] per j-block; exp on ACT/DVE -> st (fp8).

    `chunks` are PE-work closures of the previous batch, interleaved
    between score blocks to fill the PE while exp paces the at-ring."""
    pt_h, pwct_h = s.pt_h[bi], s.pwct_h[bi]

    st = s.big.tile([128, NB, 1024], fp8, tag="st")
    st_u8 = st.bitcast(u8)
    ci = 0
    for jb in range(NB):
        at = s.ps_at.tile([128, 1024], f32, tag="at")
        nc.tensor.matmul(at[:, 0:512], pt_h[:, jb, :],
                         pwct_h[:, 0:4, :], start=True, stop=True)
        nc.tensor.matmul(at[:, 512:1024], pt_h[:, jb, :],
                         pwct_h[:, 4:8, :], start=True, stop=True)
        with s.tc.high_priority(offset=50):
            if EXP_SPLIT[jb] == "a":
                nc.scalar.activation(st[:, jb, :], at, Exp,
                                     bias=s.vcols[:, bi, jb:jb + 1])
            else:
                nc.vector.tensor_scalar(st_u8[:, jb, :], at, M_SCHR,
                                        s.vcols[:, bi, 8 + jb:9 + jb], Mult, Add)
        if ci < len(chunks):
            chunks[ci]()
            ci += 1
    for c in chunks[ci:]:
        c()
    s.st[bi] = st


def _attn_gates_chunks(nc, s, bi):
    """PE-work closures for attention + gates of batch bi (with their
    attached ACT/DVE/GPSIMD consumers), interleaved into the next batch's
    score stream.  The tail chain per half-batch is:
    den -> recip (DVE) -> partition_broadcast (GPSIMD) -> itr-norm (DVE)
    -> gate MMs -> tanh (ACT) -> fused output (DVE) -> DMA."""
    st, pn_f8, pt_h = s.st[bi], s.pn_f8[bi], s.pt_h[bi]
    h = {}

    def den_chunk(c):
        def go():
            den_ps = s.ps_di.tile([128, 512], f32, tag="di")
            cs = slice(c * 512, (c + 1) * 512)
            for t in range(NB // 2):
                nc.tensor.matmul(den_ps[0:1, :], s.ones_f8,
                                 st[:, 2 * t:2 * t + 2, cs],
                                 perf_mode=DR, start=(t == 0), stop=(t == 3))
            if c == 0:
                recip_f = s.work.tile([1, 1024], f32, tag="recip")
                bc_f = s.work.tile([128, NB, 128], f32, tag="bc_f")
                h["recip_f"], h["bc_f"] = recip_f, bc_f
            cs2 = slice(c * 512, (c + 1) * 512)
            nc.vector.reciprocal_approx_fast(h["recip_f"][:, cs2], den_ps[0:1, :])
            nc.gpsimd.partition_broadcast(
                h["bc_f"][:, 4 * c:4 * c + 4, :], h["recip_f"][:, cs2])
        return go

    def itr_chunk(c):
        def go():
            itr_ps = s.ps_di.tile([128, 512], f32, tag="di")
            cs = slice(c * 512, (c + 1) * 512)
            for t in range(NB // 2):
                nc.tensor.matmul(itr_ps, pn_f8[:, 2 * t:2 * t + 2, :],
                                 st[:, 2 * t:2 * t + 2, cs], perf_mode=DR,
                                 start=(t == 0), stop=(t == 3))
            if c == 0:
                itrt = s.work.tile([128, NB, 128], bf16, tag="itrt_h")
                h["itrt"] = itrt
            with nc.allow_low_precision(reason="bf16 itr"):
                nc.vector.tensor_mul(
                    h["itrt"][:, 4 * c:4 * c + 4, :],
                    itr_ps.rearrange("p (t d) -> p t d", t=4),
                    h["bc_f"][:, 4 * c:4 * c + 4, :])
        return go

    def gate_chunk(c, g):
        def go():
            if c == 0 and g == 0:
                zt = s.work.tile([128, NB, 128], bf16, tag="zt")
                tr = s.work.tile([128, NB, 128], bf16, tag="tr")
                tf = s.work.tile([128, NB, 128], bf16, tag="tf")
                h["zt"], h["tr"], h["tf"] = zt, tr, tf
            hb = slice(4 * c, 4 * c + 4)
            g_ps = s.ps_g.tile([128, 512], f32, tag="g")
            nc.tensor.matmul(g_ps, s.w_h[:, g, 0, :], pt_h[:, hb, :],
                             start=True, stop=False)
            nc.tensor.matmul(g_ps, s.w_h[:, g, 1, :], h["itrt"][:, hb, :],
                             start=False, stop=True)
            if g == 0:
                nc.scalar.activation(h["zt"][:, hb, :], g_ps, Tanh,
                                     bias=s.b_cols[:, 0:1])
            else:
                key = "tr" if g == 1 else "tf"
                nc.scalar.activation(h[key][:, hb, :], g_ps, Tanh,
                                     bias=s.bhalf[:, g:g + 1], scale=0.5)
        return go

    def u_chunk(c):
        def go():
            hb = slice(4 * c, 4 * c + 4)
            if c == 0:
                u = s.work.tile([128, NB, 128], bf16, tag="u")
                h["u"] = u
            nc.vector.scalar_tensor_tensor(h["u"][:, hb, :], h["tr"][:, hb, :],
                                           1.0, pt_h[:, hb, :], Add, Mult)
        return go

    def out_chunk(c):
        def go():
            hb = slice(4 * c, 4 * c + 4)
            # 2*out^T = (1+tr)*P^T + (1+tf)*z; host halves + un-transposes
            if c == 0:
                w2t = s.work.tile([128, NB, 128], bf16, tag="w2t")
                out_th = s.work.tile([128, NB, 128], bf16, tag="out_th")
                h["w2t"], h["out_th"] = w2t, out_th
            nc.vector.scalar_tensor_tensor(h["w2t"][:, hb, :], h["tf"][:, hb, :],
                                           1.0, h["zt"][:, hb, :], Add, Mult)
            nc.vector.tensor_add(h["out_th"][:, hb, :], h["u"][:, hb, :],
                                 h["w2t"][:, hb, :])
            nc.sync.dma_start(
                out=s.out[bi].rearrange("d (t m) -> d t m", t=NB)[:, hb, :],
                in_=h["out_th"][:, hb, :])
        return go

    def fuse(*fns):
        def go():
            for f in fns:
                f()
        return go

    return [
        den_chunk(0),
        itr_chunk(0),
        den_chunk(1),
        itr_chunk(1),
        gate_chunk(0, 0),
        fuse(gate_chunk(0, 1), gate_chunk(0, 2)),
        fuse(u_chunk(0), gate_chunk(1, 0), out_chunk(0)),
        fuse(gate_chunk(1, 1), gate_chunk(1, 2)),
        fuse(u_chunk(1), out_chunk(1)),
    ]


def _body(nc, tc, ctx):
    s = _State()
    s.tc = tc
    s.P_t = nc.dram_tensor("P_t", [BPC, D, N], bf16, kind="ExternalInput")
    s.P_wct = nc.dram_tensor("P_wct", [BPC, D, N], bf16, kind="ExternalInput")
    s.P_f8 = nc.dram_tensor("P_f8", [BPC, N, D], fp8, kind="ExternalInput")
    w_att = nc.dram_tensor("w_itr_att", [3 * D], f32, kind="ExternalInput")
    w1 = nc.dram_tensor("w1", [2 * D, D], f32, kind="ExternalInput")
    w2 = nc.dram_tensor("w2", [2 * D, D], f32, kind="ExternalInput")
    w3 = nc.dram_tensor("w3", [2 * D, D], f32, kind="ExternalInput")
    b1 = nc.dram_tensor("b1", [D], f32, kind="ExternalInput")
    b2 = nc.dram_tensor("b2", [D], f32, kind="ExternalInput")
    b3 = nc.dram_tensor("b3", [D], f32, kind="ExternalInput")
    s.vcols_d = nc.dram_tensor("vcols", [128, BPC * 16], f32, kind="ExternalInput")
    s.out = nc.dram_tensor("out", [BPC, D, N], bf16, kind="ExternalOutput")

    singles = ctx.enter_context(tc.tile_pool(name="singles", bufs=1))
    s.pt_h, s.pwct_h, s.pn_f8, s.st = {}, {}, {}, {}

    s.work = ctx.enter_context(tc.tile_pool(name="work", bufs=3))
    s.ld = ctx.enter_context(tc.tile_pool(name="ld", bufs=3))
    s.big = ctx.enter_context(tc.tile_pool(name="big", bufs=3))
    # PSUM: ps_at 2x[128,1024] (4) + ps_di 2x[128,512] (2) + ps_g 2x[128,512]
    # (2) = 8 banks
    s.ps_at = ctx.enter_context(tc.tile_pool(name="ps_at", bufs=2, space="PSUM"))
    s.ps_di = ctx.enter_context(tc.tile_pool(name="ps_di", bufs=2, space="PSUM"))
    s.ps_g = ctx.enter_context(tc.tile_pool(name="ps_g", bufs=2, space="PSUM"))
    vc_t = singles.tile([128, BPC, 16], f32)
    nc.sync.dma_start(out=vc_t,
                      in_=s.vcols_d[0:128, :].rearrange("p (b c) -> p b c", b=BPC))
    s.vcols = vc_t
    _load(nc, s, 0)

    # ---- constants ----
    ones2_f = singles.tile([1, 2], f32)
    nc.vector.memset(ones2_f, 1.0)

    ones_f = singles.tile([128, 1], f32)
    nc.vector.memset(ones_f, 1.0)
    ones_rf = singles.tile([1, 128], f32)
    nc.vector.memset(ones_rf, 1.0)
    # DoubleRow denominator lhsT: Ko-dim byte step must be %16 == 0
    ones_f8_pad = singles.tile([128, 2, 16], fp8)
    nc.vector.tensor_copy(ones_f8_pad[:, 0, 0:1], ones_f)
    nc.vector.tensor_copy(ones_f8_pad[:, 1, 0:1], ones_f)
    s.ones_f8 = ones_f8_pad[:, :, 0:1]

    # Gate weights: per-gate stationaries [d, 128]: rows 0:128 (vs P^T) and
    # rows 128:256 (vs itr^T).
    wstage = singles.tile([128, 3, 2, 128], f32)
    for gi, w in enumerate((w1, w2, w3)):
        nc.gpsimd.dma_start(out=wstage[:, gi, 0, :], in_=w[0:128, :])
        nc.gpsimd.dma_start(out=wstage[:, gi, 1, :], in_=w[128:256, :])
    s.w_h = singles.tile([128, 3, 2, 128], bf16)
    nc.vector.tensor_copy(s.w_h, wstage)

    # biases as per-partition columns (rows -> columns via K=1 matmuls)
    bstage = singles.tile([1, 3, 128], f32)
    for gi, bvec in enumerate((b1, b2, b3)):
        nc.gpsimd.dma_start(out=bstage[:, gi, :],
                            in_=bvec.rearrange("(o p) -> o p", o=1))
    b_ps = s.ps_g.tile([128, 512], f32, tag="g")
    for gi in range(3):
        nc.tensor.matmul(b_ps[:, 2 * gi:2 * gi + 2], bstage[:, gi, :], ones2_f,
                         start=True, stop=True)
    s.b_cols = singles.tile([128, 3], f32)
    nc.vector.tensor_copy(s.b_cols, b_ps[:, 0:6:2])
    s.bhalf = singles.tile([128, 3], f32)
    nc.vector.tensor_scalar_mul(s.bhalf, s.b_cols, 0.5)

    # Pipeline: batch bi's scores interleave the PE chunks of bi-1.
    _load(nc, s, 1)
    _load_f8(nc, s, 0)
    chunks = []
    for bi in range(BPC):
        if bi + 2 < BPC:
            _load(nc, s, bi + 2)
        if bi + 1 < BPC:
            _load_f8(nc, s, bi + 1)
        _scores(nc, s, bi, chunks)
        chunks = _attn_gates_chunks(nc, s, bi)
    for c in chunks:
        c()


_NC_CACHE = {}


def _get_nc():
    if "nc" not in _NC_CACHE:
        nc = bacc.Bacc(None)
        with tile.TileContext(nc) as tc:
            with ExitStack() as ctx:
                _body(nc, tc, ctx)
        nc.finalize()
        _NC_CACHE["nc"] = nc
    return _NC_CACHE["nc"]


def _prep_inputs(inputs):
    P = np.asarray(inputs["P"], dtype=np.float32)
    watt = np.asarray(inputs["w_itr_att"], dtype=np.float32)
    wa, wc = watt[:D], watt[2 * D:]
    in_maps = []
    for c in range(NCORES):
        shard = np.ascontiguousarray(P[c * BPC:(c + 1) * BPC])  # [BPC, N, D]
        # transposed, column-permuted: cols (t, m) <-> n = m*NB + t
        pt = shard.transpose(0, 2, 1).reshape(BPC, D, 128, NB)
        pt = np.ascontiguousarray(pt.transpose(0, 1, 3, 2)).reshape(BPC, D, N)
        v = (shard @ wa).reshape(BPC, 128, NB)        # v[bi, p, t], n = p*NB+t
        vcols = np.concatenate([v + D_ACT, M_SCHR * v + C_SCHR], axis=2)
        vcols = np.ascontiguousarray(vcols.transpose(1, 0, 2)).reshape(128, BPC * 16)
        m = {
            "P_t": pt.astype(ml_dtypes.bfloat16),
            "vcols": vcols.astype(np.float32),
            "P_wct": (wc[None, :, None] * pt).astype(ml_dtypes.bfloat16),
            "P_f8": shard.astype(ml_dtypes.float8_e4m3),
            "w_itr_att": np.asarray(inputs["w_itr_att"], dtype=np.float32),
            "w1": np.asarray(inputs["w1"], dtype=np.float32),
            "w2": np.asarray(inputs["w2"], dtype=np.float32),
            "w3": np.asarray(inputs["w3"], dtype=np.float32),
            "b1": np.asarray(inputs["b1"], dtype=np.float32),
            "b2": np.asarray(inputs["b2"], dtype=np.float32),
            "b3": np.asarray(inputs["b3"], dtype=np.float32),
        }
        in_maps.append(m)
    return in_maps


def _run(inputs, _retries=2, **kw):
    nc = _get_nc()
    in_maps = _prep_inputs(inputs)
    import time
    for attempt in range(_retries + 1):
        try:
            res = run_bass_kernel_spmd(nc, in_maps,
                                       core_ids=list(range(NCORES)), **kw)
            break
        except Exception:  # wedged device from a prior aborted run
            if attempt == _retries:
                raise
            time.sleep(20)
    shards = []
    for r in res.results:
        a = np.asarray(r["out"]).astype(np.float32)        # [BPC, D, (t, m)]
        a = a.reshape(BPC, D, NB, 128).transpose(0, 3, 2, 1)  # -> [BPC, m, t, D]
        shards.append(a.reshape(BPC, N, D))                # n = m*NB + t
    outp = 0.5 * np.concatenate(shards, axis=0)
    return outp.astype(np.float32), res


def kernel(**inputs):
    out, _ = _run(inputs)
    return out
